# revision 42
# baseline (speedup 1.0000x reference)
"""DiffMHA (differential multi-head attention) block on 8 TRN2 NeuronCores.

Problem: B=4, L=1024, D=1024, H=16 heads (DH=64). Three input streams
(e_v, e_a0, e_a1); Q/K projections per stream, scores summed across
streams, causal-masked softmax, context from the v-stream values,
out-projection + residual + LayerNorm.

Sharding: (batch, head-half) -> 8 cores. Core c handles batch c//2 and
heads (c%2)*8 .. (c%2)*8+8. Each core computes its 8 heads' Q/K/V
projections, scores + softmax + context. After each channel fold's
context is ready it is AllGathered (bf16) within the batch pair,
overlapped with the next fold's compute; every core then runs the FULL
out-projection + residual + LayerNorm locally (no exposed collective at
the end) and the host keeps each core's own half of the rows.

Performance structure:
- Channel-major activations; bf16 matmul operands; fp32 PSUM.
- Causal skip: q-half 0 only computes k-tiles 0-3; masking is a
  multiplicative 0/1 bf16 mask applied AFTER exp on diagonal tiles only.
- Softmax 1/sum: Scalar-engine copy of the sum row to SBUF, then a
  fast approximate reciprocal — keeps the slow multi-pass reciprocal
  off the in-order Vector FIFO that gates the PE.
- Next-fold projection matmuls are emitted BETWEEN attention groups so
  the in-order PE queue never drains on the exp->mask->ctx chain.
- Out-projection runs in three phases (folds 0-1, fold 2, fold 3)
  accumulating into bf16 SBUF; running-accumulator adds are folded in
  via identity matmuls (PE) + Scalar copies instead of Vector adds.
- Each fold's AllGather is split into two L-halves so exchanges start
  half-a-fold earlier; only fold 3's second half is tail-exposed.
"""

import os
import sys
import types

import ml_dtypes
import numpy as np

B, L, D, H = 4, 1024, 1024, 16
DH = D // H
HPC = H // 2  # heads per core
C = HPC * DH  # channels per core (512)
SCALE = float(1.0 / np.sqrt(DH))
EPS = 1e-12
NCORES = 8
BF16 = ml_dtypes.bfloat16


def _install_ntff_hook():
    """Recreate antenv.axon_hooks (absent in this image) so
    run_bass_kernel_spmd(trace=True) can capture NTFF profiles."""
    if "antenv.axon_hooks" in sys.modules:
        return
    try:
        from trn_agent_boot.trn_boot import _ntff_profile_via_ctypes

        hook = _ntff_profile_via_ctypes("/opt/axon/libaxon_pjrt.so")
    except Exception:
        hook = None
    mod = types.ModuleType("antenv.axon_hooks")
    mod.get_axon_ntff_profile_hook = lambda: hook
    mod.set_axon_ntff_profile_hook = lambda h: None
    sys.modules["antenv.axon_hooks"] = mod


_install_ntff_hook()

import concourse.bass as bass  # noqa: E402
import concourse.mybir as mybir  # noqa: E402
import concourse.tile as tile  # noqa: E402
from concourse import bacc  # noqa: E402
from concourse.bass_utils import run_bass_kernel_spmd  # noqa: E402

F32 = mybir.dt.float32
BF = mybir.dt.bfloat16
AF = mybir.ActivationFunctionType
ALU = mybir.AluOpType

_NC_CACHE = {}
LAST_RESULT = None

NQF = C // 128  # 4 channel folds per stream (2 heads each)
NLT = L // 128  # 8 l-tiles
NDT = D // 128  # 8 d-tiles (contraction)
NKT = L // 128  # 8 k-tiles
STREAMS = ("v", "a0", "a1")
PAIRS = [[0, 1], [2, 3], [4, 5], [6, 7]]


def _included_kts(qh):
    """k-tiles that are not fully causally masked for this q-half."""
    return range(4) if qh == 0 else range(NKT)


def _diag_pattern(qh, kt):
    """Index of the 0/1 triangular mask pattern, or None if the tile is
    fully visible (no masking needed)."""
    if qh == 0:
        return kt  # tiles 0-3 straddle the diagonal
    return kt - 4 if kt >= 4 else None


def build_nc():
    nc = bacc.Bacc("TRN2", target_bir_lowering=False, debug=False, num_devices=NCORES)

    # ---- DRAM parameters (per-core shards, host-prepped) ----
    xt = {s: nc.declare_dram_parameter(f"xt_{s}", [D, L], BF, isOutput=False) for s in STREAMS}
    # W fold-sliced on host: [NQF, D, 128]
    wq = {s: nc.declare_dram_parameter(f"wq_{s}", [NQF, D, 128], BF, isOutput=False) for s in STREAMS}
    wk = {s: nc.declare_dram_parameter(f"wk_{s}", [NQF, D, 128], BF, isOutput=False) for s in STREAMS}
    wv = nc.declare_dram_parameter("wv", [D, C], BF, isOutput=False)
    # full Wout, rows grouped [pair-rank j][fold f][128]: [128, 2*NQF, D]
    wout = nc.declare_dram_parameter("wout", [128, 2 * NQF, D], BF, isOutput=False)
    bq = {s: nc.declare_dram_parameter(f"bq_{s}", [C], F32, isOutput=False) for s in STREAMS}
    bk = {s: nc.declare_dram_parameter(f"bk_{s}", [C], F32, isOutput=False) for s in STREAMS}
    bv = nc.declare_dram_parameter("bv", [1, C], BF, isOutput=False)
    bout_full = nc.declare_dram_parameter("bout_full", [1, D], BF, isOutput=False)
    # 0/1 exp-mask "window" tensor: E[kl, t] = (kl <= t - 384); pattern i
    # is the 512-wide slice starting at 384 - 128*i.
    trimask = nc.declare_dram_parameter("trimask", [128, 896], BF, isOutput=False)
    ev_res = nc.declare_dram_parameter("ev_res", [L, D], BF, isOutput=False)
    ident = nc.declare_dram_parameter("ident", [128, 128], BF, isOutput=False)
    gamma = nc.declare_dram_parameter("gamma", [1, D], BF, isOutput=False)
    beta = nc.declare_dram_parameter("beta", [1, D], BF, isOutput=False)
    out = nc.declare_dram_parameter("out", [L, D], F32, isOutput=True)

    with tile.TileContext(nc) as tc:
        with (
            tc.tile_pool(name="persist", bufs=1) as persist,
            tc.tile_pool(name="xtp", bufs=1) as xtp,
            tc.tile_pool(name="wf", bufs=10) as wf,
            tc.tile_pool(name="qkf", bufs=2) as qkf,
            tc.tile_pool(name="small", bufs=4) as small,
            tc.tile_pool(name="attn", bufs=3) as attn_pool,
            tc.tile_pool(name="stage", bufs=1) as stage_pool,
            tc.tile_pool(name="ln", bufs=3) as ln_pool,
            tc.tile_pool(name="proj_ps", bufs=2, space="PSUM") as proj_ps,
            tc.tile_pool(name="sc_ps", bufs=4, space="PSUM") as sc_ps,
            tc.tile_pool(name="ctx_ps", bufs=2, space="PSUM") as ctx_ps,
            tc.tile_pool(name="dram", bufs=1, space="DRAM") as dram,
        ):
            # ---- persistent SBUF tensors ----
            vnat = persist.tile([128, NLT, HPC, DH + 1], BF, tag="vnat")
            allctx = persist.tile([128, 2, NQF, L], BF, tag="allctx")
            out_acc = persist.tile([128, NLT, D], BF, tag="outacc")
            trimask_sb = persist.tile([128, 896], BF, tag="trimask")
            ones_b = persist.tile([1, 128], BF, tag="ones")
            gb_bc = persist.tile([128, 2, D], BF, tag="gbbc")
            bout_sb = persist.tile([1, D], BF, tag="boutsb")
            bv_sb = persist.tile([1, C], BF, tag="bvsb")
            wout_sb = persist.tile([128, 2 * NQF, D], BF, tag="woutsb")
            eps_sb = persist.tile([128, 1], F32, tag="eps")
            wv_sb = persist.tile([128, NDT, C], BF, tag="wvsb")
            bq_sb = {
                s: persist.tile([128, NQF], F32, tag=f"bq{s}", name=f"bq_sb_{s}")
                for s in STREAMS
            }
            bk_sb = {
                s: persist.tile([128, NQF], F32, tag=f"bk{s}", name=f"bk_sb_{s}")
                for s in STREAMS
            }

            nc.vector.memset(ones_b[:, :], 1.0)
            nc.vector.memset(eps_sb[:, :], EPS)
            # V ones-column (feeds the softmax-sum rows)
            nc.vector.memset(vnat[:, :, :, DH : DH + 1], 1.0)

            # ---- startup DMAs: wv + xt_v first so the PE starts ASAP ----
            xt_sb = {}
            for s in STREAMS:
                xt_sb[s] = xtp.tile(
                    [128, NDT, L], BF, tag=f"xt{s}", name=f"xt_sb_{s}"
                )
            nc.sync.dma_start(
                out=wv_sb[:, :, :],
                in_=wv[:, :].rearrange("(dt p) c -> p dt c", p=128),
            )
            for dt in range(NDT):
                nc.sync.dma_start(
                    out=xt_sb["v"][:, dt, :], in_=xt["v"][dt * 128 : (dt + 1) * 128, :]
                )
            nc.sync.dma_start(out=bv_sb[:, :], in_=bv[:, :])
            for s in STREAMS:
                nc.sync.dma_start(
                    out=bq_sb[s][:, :], in_=bq[s][:].rearrange("(f p) -> p f", p=128)
                )
                nc.sync.dma_start(
                    out=bk_sb[s][:, :], in_=bk[s][:].rearrange("(f p) -> p f", p=128)
                )
            nc.sync.dma_start(out=trimask_sb[:, :], in_=trimask[:, :])
            ident_sb = persist.tile([128, 128], BF, tag="ident")
            nc.sync.dma_start(out=ident_sb[:, :], in_=ident[:, :])

            # fold-weight prefetch helper (wf holds 2 folds)
            w_tiles = {}

            def emit_wdma(f):
                tiles = {}
                for s in STREAMS:
                    wq_t = wf.tile([128, NDT, 128], BF, tag="w", name=f"wq_{s}{f}")
                    wk_t = wf.tile([128, NDT, 128], BF, tag="w", name=f"wk_{s}{f}")
                    nc.sync.dma_start(
                        out=wq_t[:, :, :],
                        in_=wq[s][f, :, :].rearrange("(dt p) c -> p dt c", p=128),
                    )
                    nc.sync.dma_start(
                        out=wk_t[:, :, :],
                        in_=wk[s][f, :, :].rearrange("(dt p) c -> p dt c", p=128),
                    )
                    tiles[s] = (wq_t, wk_t)
                w_tiles[f] = tiles

            emit_wdma(0)

            # ---- V projection (natural [l, c] layout + ones column) ----
            for lf in range(NLT):
                ps = proj_ps.tile([128, C], F32, tag="proj")
                for dt in range(NDT):
                    nc.tensor.matmul(
                        ps[:, :],
                        xt_sb["v"][:, dt, lf * 128 : (lf + 1) * 128],
                        wv_sb[:, dt, :],
                        start=(dt == 0),
                        stop=False,
                    )
                # + bias via ones-row rank-1 update
                nc.tensor.matmul(
                    ps[:, :],
                    ones_b[:, :],
                    bv_sb[:, :],
                    start=False,
                    stop=True,
                )
                nc.scalar.copy(vnat[:, lf, :, 0:DH], ps[:, :])

            # remaining embeddings + late-needed tensors
            for s in ("a0", "a1"):
                for dt in range(NDT):
                    nc.sync.dma_start(
                        out=xt_sb[s][:, dt, :],
                        in_=xt[s][dt * 128 : (dt + 1) * 128, :],
                    )
            emit_wdma(1)
            nc.sync.dma_start(out=wout_sb[:, :, :], in_=wout[:, :, :])
            nc.sync.dma_start(out=bout_sb[:, :], in_=bout_full[:, :])
            gsb = small.tile([1, D], BF, tag="gsb", bufs=1)
            bsb = small.tile([1, D], BF, tag="bsb", bufs=1)
            nc.sync.dma_start(out=gsb[:, :], in_=gamma[:, :])
            nc.sync.dma_start(out=bsb[:, :], in_=beta[:, :])
            nc.gpsimd.partition_broadcast(gb_bc[:, 0, :], gsb[:, :])
            nc.gpsimd.partition_broadcast(gb_bc[:, 1, :], bsb[:, :])

            # AllGather DRAM staging: one [128, L] per fold 0-2, two halves
            # for fold 3 so its exchange starts earlier.
            ag_in = {}
            ag_out = {}
            for key in ("0a", "0b", "1a", "1b", "2a", "2b", "3a", "3b"):
                cols = 512
                ag_in[key] = dram.tile(
                    [128, cols], BF, name=f"ag_in{key}", tag=f"agin{key}"
                )
                ag_out[key] = dram.tile(
                    [2, 128, cols], BF, name=f"ag_out{key}", tag=f"agout{key}"
                )

            def emit_ag(key, f, csl):
                nc.gpsimd.collective_compute(
                    "AllGather",
                    ALU.bypass,
                    replica_groups=PAIRS,
                    ins=[ag_in[key].opt()],
                    outs=[ag_out[key].opt()],
                )
                nc.sync.dma_start(
                    out=allctx[:, :, f, csl],
                    in_=ag_out[key][:, :, :].rearrange("j p l -> p j l"),
                )

            # out-projection phase helper: accumulate fold range into ops;
            # stop=True on the last matmul unless the caller appends more.
            def outproj_mms(ops, folds, lt, dh_i, last_stops):
                lsl = slice(lt * 128, (lt + 1) * 128)
                dsl = slice(dh_i * 512, (dh_i + 1) * 512)
                n = 0
                total = 2 * len(folds)
                for j in range(2):
                    for cf in folds:
                        n += 1
                        nc.tensor.matmul(
                            ops[:, :],
                            allctx[:, j, cf, lsl],
                            wout_sb[:, j * NQF + cf, dsl],
                            start=(n == 1),
                            stop=(last_stops and n == total),
                        )
                return dsl, lsl

            # ---- fold-major schedule with interleaved emission: the PE
            #      queue is in-order, so next-fold projection matmuls are
            #      emitted BETWEEN this fold's attention groups to cover
            #      the exp->mask->ctx pipeline drains. ----
            def emit_proj(f, streams_sel, qtf, ktf):
                for s in streams_sel:
                    wq_t, wk_t = w_tiles[f][s]
                    for which, w_t, b_t, store in (
                        ("q", wq_t, bq_sb[s], qtf),
                        ("k", wk_t, bk_sb[s], ktf),
                    ):
                        dst = qkf.tile(
                            [128, L], BF, tag=f"{which}t{s}", name=f"{which}t_{s}{f}"
                        )
                        for lh in range(2):
                            ps = proj_ps.tile([128, 512], F32, tag="proj")
                            for dt in range(NDT):
                                nc.tensor.matmul(
                                    ps[:, :],
                                    w_t[:, dt, :],
                                    xt_sb[s][:, dt, lh * 512 : (lh + 1) * 512],
                                    start=(dt == 0),
                                    stop=(dt == NDT - 1),
                                )
                            nc.scalar.activation(
                                dst[:, lh * 512 : (lh + 1) * 512],
                                ps[:, :],
                                AF.Identity,
                                bias=b_t[:, f : f + 1],
                            )
                        store[s] = dst

            def emit_group(f, qh, qtf, ktf, ctx_stage):
                """Both heads of the fold for one q-half, with the two
                heads' 64-partition score matmuls emitted back-to-back:
                they target disjoint PE row groups (base partitions 0/64)
                and different PSUM banks, so the array runs them
                concurrently."""
                qsl = slice(qh * 512, (qh + 1) * 512)
                kts = list(_included_kts(qh))
                cps = {
                    hh: ctx_ps.tile(
                        [DH + 1, 512], F32, tag="ctx", name=f"cps{hh}"
                    )
                    for hh in range(2)
                }
                for kt_i in kts:
                    ksl = slice(kt_i * 128, (kt_i + 1) * 128)
                    sps = {
                        hh: sc_ps.tile([128, 512], F32, tag="sc", name=f"sps{hh}")
                        for hh in range(2)
                    }
                    for i, s in enumerate(STREAMS):
                        for hh in range(2):
                            p0 = hh * 64
                            nc.tensor.matmul(
                                sps[hh][:, :],
                                ktf[s][p0 : p0 + 64, ksl],
                                qtf[s][p0 : p0 + 64, qsl],
                                start=(i == 0),
                                stop=(i == 2),
                            )
                    pat = _diag_pattern(qh, kt_i)
                    for hh in range(2):
                        attn_sb = attn_pool.tile([128, 512], BF, tag="attn")
                        nc.scalar.activation(
                            attn_sb[:, :], sps[hh][:, :], AF.Exp, scale=SCALE
                        )
                        if pat is not None:
                            off = 384 - 128 * pat
                            nc.vector.tensor_mul(
                                attn_sb[:, :],
                                attn_sb[:, :],
                                trimask_sb[:, off : off + 512],
                            )
                        nc.tensor.matmul(
                            cps[hh][:, :],
                            vnat[:, kt_i, 2 * f + hh, :],
                            attn_sb[:, :],
                            start=(kt_i == kts[0]),
                            stop=(kt_i == kts[-1]),
                        )
                # 1/sum: stage the sum row to SBUF on the Scalar engine
                # (Identity — no ACT table swap), then a fast approximate
                # reciprocal keeps the slow multi-pass reciprocal off the
                # Vector FIFO that gates the PE.
                for hh in range(2):
                    p0 = hh * 64
                    sum_sb = small.tile([1, 512], F32, tag="sumsb", bufs=1)
                    nc.scalar.copy(sum_sb[:, :], cps[hh][DH : DH + 1, :])
                    inv = small.tile([1, 512], F32, tag="inv", bufs=2)
                    nc.vector.reciprocal_approx_fast(inv[:, :], sum_sb[:, :])
                    inv_bc = small.tile([64, 512], F32, tag="invbc", bufs=2)
                    nc.gpsimd.partition_broadcast(inv_bc[:, :], inv[:, :])
                    nc.vector.tensor_mul(
                        ctx_stage[p0 : p0 + 64, qsl], cps[hh][0:DH, :], inv_bc[:, :]
                    )

            def emit_phase_a(lts):
                # out-proj contribution of folds 0-1 (+bias), residual
                # pre-added into the bf16 accumulator.
                for lt in lts:
                    lsl = slice(lt * 128, (lt + 1) * 128)
                    ev_sb = ln_pool.tile([128, D], BF, tag="ev")
                    nc.sync.dma_start(out=ev_sb[:, :], in_=ev_res[lsl, :])
                    for dh_i in range(2):
                        ops = proj_ps.tile([128, 512], F32, tag="proj")
                        dsl, _ = outproj_mms(ops, (0, 1), lt, dh_i, False)
                        nc.tensor.matmul(
                            ops[:, :],
                            ones_b[:, :],
                            bout_sb[:, dsl],
                            start=False,
                            stop=True,
                        )
                        nc.scalar.copy(out_acc[:, lt, dsl], ops[:, :])
                    nc.gpsimd.tensor_add(
                        out_acc[:, lt, :], out_acc[:, lt, :], ev_sb[:, :]
                    )

            proj_tiles = {}
            proj_tiles[0] = ({}, {})
            emit_proj(0, STREAMS, *proj_tiles[0])
            for f in range(NQF):
                qtf, ktf = proj_tiles.pop(f)
                if f + 2 < NQF:
                    emit_wdma(f + 2)
                if f < 3:
                    proj_tiles[f + 1] = ({}, {})

                ctx_stage = stage_pool.tile([128, L], BF, tag="ctxstage")
                emit_group(f, 0, qtf, ktf, ctx_stage)
                nc.sync.dma_start(out=ag_in[f"{f}a"][:, :], in_=ctx_stage[:, 0:512])
                emit_ag(f"{f}a", f, slice(0, 512))
                if f != 3:
                    emit_proj(f + 1, ("v", "a0"), *proj_tiles[f + 1])
                emit_group(f, 1, qtf, ktf, ctx_stage)
                nc.sync.dma_start(out=ag_in[f"{f}b"][:, :], in_=ctx_stage[:, 512:1024])
                emit_ag(f"{f}b", f, slice(512, 1024))
                if f != 3:
                    emit_proj(f + 1, ("a1",), *proj_tiles[f + 1])
                if f == 2:
                    emit_phase_a(range(NLT))

            # phase B: fold-3 contribution + LayerNorm
            for lt in range(NLT):
                lsl = slice(lt * 128, (lt + 1) * 128)
                x_sb = ln_pool.tile([128, D], F32, tag="x")
                for dh_i in range(2):
                    ops = proj_ps.tile([128, 512], F32, tag="proj")
                    dsl, _ = outproj_mms(ops, (2, 3), lt, dh_i, False)
                    nc.tensor.matmul(
                        ops[:, :],
                        ident_sb[:, :],
                        out_acc[:, lt, dsl],
                        start=False,
                        stop=True,
                    )
                    nc.scalar.copy(x_sb[:, dsl], ops[:, :])
                stats = small.tile([128, 2, 6], F32, tag="stats")
                nc.vector.bn_stats(out=stats[:, 0, :], in_=x_sb[:, 0:512])
                nc.vector.bn_stats(out=stats[:, 1, :], in_=x_sb[:, 512:1024])
                mv = small.tile([128, 2], F32, tag="mv")
                nc.vector.bn_aggr(out=mv[:, :], in_=stats[:, :, :])
                std = small.tile([128, 1], F32, tag="std")
                nc.scalar.activation(std[:, :], mv[:, 1:2], AF.Sqrt, bias=eps_sb[:, :])
                rstd = small.tile([128, 1], F32, tag="rstd")
                nc.vector.reciprocal(rstd[:, :], std[:, :])
                negmb = small.tile([128, 1], F32, tag="negmb")
                nc.vector.scalar_tensor_tensor(
                    negmb[:, :],
                    mv[:, 0:1],
                    -1.0,
                    rstd[:, :],
                    op0=ALU.mult,
                    op1=ALU.mult,
                )
                nc.scalar.activation(
                    x_sb[:, :],
                    x_sb[:, :],
                    AF.Identity,
                    bias=negmb[:, :],
                    scale=rstd[:, :],
                )
                # alternate gamma/beta between Vector and GpSimd so the
                # Vector FIFO (stats + adds) isn't the lone tail engine
                eng = nc.vector if lt % 2 == 0 else nc.gpsimd
                eng.tensor_mul(x_sb[:, :], x_sb[:, :], gb_bc[:, 0, :])
                eng.tensor_add(x_sb[:, :], x_sb[:, :], gb_bc[:, 1, :])
                nc.sync.dma_start(out=out[lsl, :], in_=x_sb[:, :])

    nc.compile()
    return nc


def _get_nc():
    if "nc" not in _NC_CACHE:
        _NC_CACHE["nc"] = build_nc()
    return _NC_CACHE["nc"]


def kernel(
    e_v, e_a0, e_a1, Wqv, bqv, Wkv, bkv, Wvv, bvv,
    Wqa0, bqa0, Wka0, bka0, Wqa1, bqa1, Wka1, bka1,
    Wout, bout, ln_gamma, ln_beta, attn_mask,
):
    global LAST_RESULT
    f = np.asarray
    e_v, e_a0, e_a1 = f(e_v), f(e_a0), f(e_a1)
    attn_mask = f(attn_mask)
    c32 = lambda a: np.ascontiguousarray(a, dtype=np.float32)
    cbf = lambda a: np.ascontiguousarray(np.asarray(a, dtype=np.float32).astype(BF16))

    wq_full = {"v": f(Wqv), "a0": f(Wqa0), "a1": f(Wqa1)}
    wk_full = {"v": f(Wkv), "a0": f(Wka0), "a1": f(Wka1)}
    bq_full = {"v": f(bqv), "a0": f(bqa0), "a1": f(bqa1)}
    bk_full = {"v": f(bkv), "a0": f(bka0), "a1": f(bka1)}

    xts = {}
    evs = {}
    for b in range(B):
        xts[b] = {
            "v": cbf(e_v[b].T),
            "a0": cbf(e_a0[b].T),
            "a1": cbf(e_a1[b].T),
        }
        evs[b] = cbf(e_v[b])

    # 0/1 exp-mask window from the input mask (scores^T [k, q] layout):
    # E[kl, t] = visibility of k-row kl vs q-col (t - 384); pattern i is
    # the 512-wide slice at offset 384 - 128*i.
    vis = (attn_mask[0, 0] > -0.5).astype(np.float32)  # [q, k] visible=1
    trimask_np = np.zeros((128, 896), dtype=np.float32)
    trimask_np[:, 384:896] = vis[0:512, 0:128].T
    trimask_np = cbf(trimask_np)

    # full Wout, rows regrouped [pair-rank j][fold f][128] -> [128, 2*NQF, D]
    wout_g = cbf(f(Wout).reshape(2 * NQF, 128, D).transpose(1, 0, 2))
    bout_g = cbf(f(bout)).reshape(1, D)

    def fold_slice(w, S):
        # [D, C] slice -> [NQF, D, 128] fold-major
        ws = np.asarray(w[:, S], dtype=np.float32)  # [D, C]
        return np.ascontiguousarray(
            ws.reshape(D, NQF, 128).transpose(1, 0, 2).astype(BF16)
        )

    ident_np = cbf(np.eye(128, dtype=np.float32))
    in_maps = []
    for c in range(NCORES):
        b, hh = c // 2, c % 2
        S = slice(hh * C, (hh + 1) * C)
        m = {}
        for s in STREAMS:
            m[f"xt_{s}"] = xts[b][s]
            m[f"wq_{s}"] = fold_slice(wq_full[s], S)
            m[f"wk_{s}"] = fold_slice(wk_full[s], S)
            m[f"bq_{s}"] = c32(bq_full[s][S])
            m[f"bk_{s}"] = c32(bk_full[s][S])
        m["wv"] = cbf(f(Wvv)[:, S])
        m["bv"] = cbf(f(bvv)[S]).reshape(1, C)
        m["wout"] = wout_g
        m["bout_full"] = bout_g
        m["trimask"] = trimask_np
        m["ev_res"] = evs[b]
        m["ident"] = ident_np
        m["gamma"] = cbf(f(ln_gamma)).reshape(1, D)
        m["beta"] = cbf(f(ln_beta)).reshape(1, D)
        in_maps.append(m)

    nc = _get_nc()
    trace = bool(os.environ.get("KERNEL_TRACE"))
    res = run_bass_kernel_spmd(
        nc, in_maps, core_ids=list(range(NCORES)), trace=trace
    )
    LAST_RESULT = res

    out_full = np.empty((B, L, D), dtype=np.float32)
    for c in range(NCORES):
        b, hh = c // 2, c % 2
        rows = slice(hh * 512, (hh + 1) * 512)
        out_full[b, rows, :] = res.results[c]["out"][rows, :]
    return out_full
